# revision 27
# baseline (speedup 1.0000x reference)
"""BFP8 dense layer (out = bfp_quant(x) @ bfp_quant(w) + bias) on 8 trn2 cores.

Sharding (hardcoded for x:(4,2048,2048) w:(2048,8192) bias:(8192,)):
  2D tensor-parallel grid: 4 row-shards of x (2048 rows each) x 2 col-shards
  of w (4096 cols each). core = r*2 + c computes out[r*2048:(r+1)*2048,
  c*4096:(c+1)*4096]. Each core quantizes its own shards locally (BFP blocks
  of 32 run along the last axis of both tensors; all shard boundaries are
  multiples of 32, so block structure matches the full-tensor flattening).

Quantization is exact vs the jax reference: per 32-block max-abs (DVE reduce
with apply_absolute_value), shared exponent via int-masked fp32 exponent
bits, q = saturating round-half-even fp32->int8 cast of x * 2^(7-e) (HW cast
verified RNE+saturating), deq = q * 2^(e-7) in bf16 (all deq values are
exactly representable in bf16, so a bf16 matmul with fp32 PSUM accumulation
reproduces the fp32 reference up to accumulation order).
"""
import os
import warnings

warnings.filterwarnings("ignore")
import numpy as np

import concourse.bass as bass
import concourse.mybir as mybir
import concourse.tile as tile
from concourse import bacc
from concourse.bass_utils import run_bass_kernel_spmd

# full problem
B, S, D, F = 4, 2048, 2048, 8192
M_FULL, K, N_FULL = B * S, D, F
# shard grid
RSH, CSH = 4, 2
M = M_FULL // RSH      # 2048 rows / core
N = N_FULL // CSH      # 4096 cols / core
MT = M // 128          # 16 m-tiles
KT = K // 128          # 16 k-tiles
NCH = 8                # n chunks per core
NC_W = N // NCH        # 512 cols per chunk
BS = 32                # bfp block size

F32 = mybir.dt.float32
BF16 = mybir.dt.bfloat16
I32 = mybir.dt.int32
I8 = mybir.dt.int8
RECIP_CONST = float(254 << 23)  # recip_bits = (254<<23) - scale_bits
INV_LN2 = float(np.float32(1.4426950408889634))  # matches device log2 lowering
LN2 = float(np.float32(0.6931471805599453))      # matches device exp2 lowering
MAGIC = 12582912.0  # 1.5 * 2**23 (RNE-to-integer magic)


def _emit_quant(nc, pool, src_f32, dst_bf16, qi8_tile, nblk, tag,
                qpass_pool=False, deq_pool=True):
    """Quantize src_f32 -> dst_bf16 (same shape), blocks of 32 on free axis.

    src/dst views must be [128, nblk*32] contiguous free. qpass_pool/deq_pool
    choose GpSimd vs DVE for the two full-size passes (engine balancing).
    """
    ma = pool.tile([128, nblk], F32, tag=f"{tag}_ma")
    xv = src_f32.rearrange("p (b e) -> p b e", e=BS)
    nc.vector.tensor_reduce(
        ma[:], xv, axis=mybir.AxisListType.X, op=mybir.AluOpType.max,
        apply_absolute_value=True,
    )
    # exponent e = floor(Ln(ma) * (1/ln2)) — replicates the device reference's
    # fp32 log2 (incl. its round-to-integer-crossing behavior) bit-exactly.
    ln_t = pool.tile([128, nblk], F32, tag=f"{tag}_ln")
    nc.scalar.activation(ln_t[:], ma[:], mybir.ActivationFunctionType.Ln)
    t = pool.tile([128, nblk], F32, tag=f"{tag}_t")
    nc.vector.tensor_single_scalar(t[:], ln_t[:], INV_LN2, op=mybir.AluOpType.mult)
    r = pool.tile([128, nblk], F32, tag=f"{tag}_r")
    nc.vector.tensor_scalar(
        r[:], t[:], MAGIC, MAGIC,
        op0=mybir.AluOpType.add, op1=mybir.AluOpType.subtract,
    )
    g = pool.tile([128, nblk], F32, tag=f"{tag}_g")
    nc.vector.tensor_tensor(g[:], r[:], t[:], op=mybir.AluOpType.is_gt)
    e_t = pool.tile([128, nblk], F32, tag=f"{tag}_e")
    nc.vector.tensor_tensor(e_t[:], r[:], g[:], op=mybir.AluOpType.subtract)
    # scale bits = (e + 120) << 23  (fp32 2^(e-7), exact)
    scale_i = pool.tile([128, nblk], I32, tag=f"{tag}_sc")
    nc.vector.tensor_scalar(
        scale_i[:], e_t[:], 120.0, 8388608.0,
        op0=mybir.AluOpType.add, op1=mybir.AluOpType.mult,
    )
    recip_i = pool.tile([128, nblk], I32, tag=f"{tag}_rc")
    nc.vector.tensor_scalar(
        recip_i[:], scale_i[:], RECIP_CONST, -1.0,
        op0=mybir.AluOpType.subtract, op1=mybir.AluOpType.mult,
    )
    # The reference divides by s_dev = device exp2(e-7), which is up to a few
    # hundred ulps off an exact power of two. Replicate: s_dev = Exp(e7*ln2)
    # (bit-matches device exp2), then effective reciprocal
    # r* = 2^(7-e) * (2 - s_dev*2^(7-e)), all exact in fp32, so that
    # RN(x*r*) == RN(x/s_dev) up to ~2^-46 coincidences.
    e7 = pool.tile([128, nblk], F32, tag=f"{tag}_e7")
    nc.vector.tensor_single_scalar(e7[:], e_t[:], 7.0, op=mybir.AluOpType.subtract)
    s_dev = pool.tile([128, nblk], F32, tag=f"{tag}_sd")
    nc.scalar.activation(s_dev[:], e7[:], mybir.ActivationFunctionType.Exp, scale=LN2)
    m_t = pool.tile([128, nblk], F32, tag=f"{tag}_m")
    nc.vector.tensor_tensor(
        m_t[:], s_dev[:], recip_i[:].bitcast(F32), op=mybir.AluOpType.mult,
    )
    c_t = pool.tile([128, nblk], F32, tag=f"{tag}_c")
    nc.vector.tensor_scalar(
        c_t[:], m_t[:], 2.0, -1.0,
        op0=mybir.AluOpType.subtract, op1=mybir.AluOpType.mult,
    )
    rstar = pool.tile([128, nblk], F32, tag=f"{tag}_rs")
    nc.vector.tensor_tensor(
        rstar[:], recip_i[:].bitcast(F32), c_t[:], op=mybir.AluOpType.mult,
    )
    scale_b = scale_i[:].bitcast(F32).unsqueeze(-1).broadcast_to((128, nblk, BS))
    recip_b = rstar[:].unsqueeze(-1).broadcast_to((128, nblk, BS))
    qv = qi8_tile.rearrange("p (b e) -> p b e", e=BS)
    qpass_eng = nc.gpsimd if qpass_pool else nc.vector
    qpass_eng.tensor_tensor(qv, xv, recip_b, op=mybir.AluOpType.mult)
    # dequant product is exact, so any correct int8->fp32->bf16 conversion
    # matches; engine choice is purely load balancing.
    deq_eng = nc.gpsimd if deq_pool else nc.vector
    deq_eng.tensor_tensor(
        dst_bf16.rearrange("p (b e) -> p b e", e=BS), qv, scale_b,
        op=mybir.AluOpType.mult,
    )


def build_nc(with_bias: bool, repeats: int = 1):
    nc = bacc.Bacc(None, target_bir_lowering=False)
    xs = nc.dram_tensor("xs", [M, K], F32, kind="ExternalInput")
    ws = nc.dram_tensor("ws", [K, N], F32, kind="ExternalInput")
    if with_bias:
        bs = nc.dram_tensor("bs", [N], F32, kind="ExternalInput")
    out = nc.dram_tensor("out", [M, N], F32, kind="ExternalOutput")

    with tile.TileContext(nc) as tc:
        with (
            tc.tile_pool(name="xt", bufs=1) as xt_pool,
            tc.tile_pool(name="wq", bufs=4) as wq_pool,
            tc.tile_pool(name="stage", bufs=2) as stage_pool,
            tc.tile_pool(name="xs_p", bufs=2) as xs_pool,
            tc.tile_pool(name="small", bufs=2) as small_pool,
            tc.tile_pool(name="outp", bufs=3) as out_pool,
            tc.tile_pool(name="psum_o", bufs=4, space="PSUM") as psum_o_pool,
            tc.tile_pool(name="const", bufs=1) as const_pool,
        ):
            bias_sb = None
            if with_bias:
                bias_sb = const_pool.tile([128, N], F32)
                bias_bcast = bass.AP(
                    tensor=bs.ap().tensor, offset=bs.ap().offset,
                    ap=[[0, 128], [1, N]],
                )
                nc.sync.dma_start(bias_sb[:], bias_bcast)

            # resident transposed quantized x: per m-tile [128 kpart, KT, 128 m]
            xT = [
                xt_pool.tile([128, KT, 128], BF16, tag=f"xT{m}", name=f"xT{m}")
                for m in range(MT)
            ]

            def emit_x_mtile(m):
                xstage = xs_pool.tile([128, K], F32, tag="xstage")
                nc.sync.dma_start(xstage[:], xs[m * 128:(m + 1) * 128, :])
                xq8 = xs_pool.tile([128, K], I8, tag="xq8", bufs=3)
                xqb = xs_pool.tile([128, K], BF16, tag="xqb", bufs=3)
                _emit_quant(nc, small_pool, xstage[:], xqb[:], xq8[:], K // BS, "x")
                # one big xbar transpose: [128 m, 2048 k] -> [128 kp, KT, 128 m]
                nc.scalar.dma_start_transpose(xT[m][:], xqb[:])

            WH = KT // 4  # k-quarters per w chunk for pipelining

            def emit_w_chunk(c, on_pool=False):
                # four independent quarter-tiles: MMs for k-quarter h only
                # depend on that quarter's quant, not the whole chunk
                wq_qs = []
                for h in range(4):
                    wq_h = wq_pool.tile([128, WH, NC_W], BF16, tag=f"wq{h}",
                                        name=f"wq_c{c}_h{h}")
                    wstage = stage_pool.tile([128, WH, NC_W], F32, tag="wstage")
                    src = bass.AP(
                        tensor=ws.ap().tensor,
                        offset=c * NC_W + h * WH * 128 * N,
                        ap=[[N, 128], [128 * N, WH], [1, NC_W]],
                    )
                    nc.sync.dma_start(wstage[:], src)
                    w8 = stage_pool.tile([128, WH, NC_W], I8, tag="w8")
                    _emit_quant(
                        nc, small_pool,
                        wstage[:].rearrange("p t n -> p (t n)"),
                        wq_h[:].rearrange("p t n -> p (t n)"),
                        w8[:].rearrange("p t n -> p (t n)"),
                        WH * NC_W // BS, "w",
                        qpass_pool=False, deq_pool=True,
                    )
                    wq_qs.append(wq_h)
                return wq_qs

            def emit_mm_block(m, c, wq):
                po = psum_o_pool.tile([128, NC_W], F32, tag="po")
                for k in range(KT):
                    nc.tensor.matmul(
                        po[:], xT[m][:, k, :], wq[k // WH][:, k % WH, :],
                        start=(k == 0), stop=(k == KT - 1),
                    )
                ob = out_pool.tile([128, NC_W], F32, tag="ob")
                if with_bias:
                    nc.vector.tensor_tensor(
                        ob[:], po[:], bias_sb[:, c * NC_W:(c + 1) * NC_W],
                        op=mybir.AluOpType.add,
                    )
                else:
                    nc.scalar.copy(ob[:], po[:])
                nc.sync.dma_start(
                    out[m * 128:(m + 1) * 128, c * NC_W:(c + 1) * NC_W], ob[:],
                )

            def emit_body():
                # x first (short path to first MM), then stream. During the
                # x-feed phase interleave chunks c0/c1/c2 (staggered) so PE
                # demand stays above the x-pipeline's production rate.
                emit_x_mtile(0)
                wq_0 = emit_w_chunk(0)
                emit_x_mtile(1)
                wqs = {0: wq_0}
                for m in range(MT):
                    emit_mm_block(m, 0, wqs[0])
                    if m == 1:
                        wqs[1] = emit_w_chunk(1, on_pool=True)
                    if m == 4:
                        wqs[2] = emit_w_chunk(2, on_pool=True)
                    if m + 2 < MT:
                        emit_x_mtile(m + 2)
                    if m >= 3:
                        emit_mm_block(m - 3, 1, wqs[1])
                    if m >= 6:
                        emit_mm_block(m - 6, 2, wqs[2])
                for m in range(MT - 3, MT):
                    emit_mm_block(m, 1, wqs[1])
                for m in range(MT - 6, MT):
                    emit_mm_block(m, 2, wqs[2])
                for c in range(3, NCH):
                    wq_c = emit_w_chunk(c)
                    for m in range(MT):
                        emit_mm_block(m, c, wq_c)

            if repeats > 1:
                with tc.For_i(0, repeats, 1):
                    emit_body()
            else:
                emit_body()

    nc.finalize()
    return nc


_NC_CACHE = {}


def _get_nc(with_bias: bool):
    if with_bias not in _NC_CACHE:
        _NC_CACHE[with_bias] = build_nc(with_bias)
    return _NC_CACHE[with_bias]


def run(x, kernel, bias, trace=False):
    x2d = np.ascontiguousarray(np.asarray(x, dtype=np.float32).reshape(M_FULL, K))
    w = np.ascontiguousarray(np.asarray(kernel, dtype=np.float32))
    b = np.asarray(bias, dtype=np.float32)
    with_bias = bool(np.any(b))
    nc = _get_nc(with_bias)

    in_maps = []
    for core in range(8):
        r, c = divmod(core, CSH)
        m = {
            "xs": x2d[r * M:(r + 1) * M, :],
            "ws": np.ascontiguousarray(w[:, c * N:(c + 1) * N]),
        }
        if with_bias:
            m["bs"] = np.ascontiguousarray(b[c * N:(c + 1) * N])
        in_maps.append(m)

    res = run_bass_kernel_spmd(nc, in_maps, core_ids=list(range(8)), trace=trace)
    full = np.empty((M_FULL, N_FULL), dtype=np.float32)
    for core in range(8):
        r, c = divmod(core, CSH)
        full[r * M:(r + 1) * M, c * N:(c + 1) * N] = res.results[core]["out"]
    return full.reshape(B, S, N_FULL), res


def kernel(x, kernel, bias):
    out, _ = run(x, kernel, bias, trace=False)
    return out


# revision 33
# speedup vs baseline: 1.0124x; 1.0124x over previous
"""BFP8 dense layer (out = bfp_quant(x) @ bfp_quant(w) + bias) on 8 trn2 cores.

Sharding (hardcoded for x:(4,2048,2048) w:(2048,8192) bias:(8192,)):
  2D tensor-parallel grid: 4 row-shards of x (2048 rows each) x 2 col-shards
  of w (4096 cols each). core = r*2 + c computes out[r*2048:(r+1)*2048,
  c*4096:(c+1)*4096]. Each core quantizes its own shards locally (BFP blocks
  of 32 run along the last axis of both tensors; all shard boundaries are
  multiples of 32, so block structure matches the full-tensor flattening).

Quantization is exact vs the jax reference: per 32-block max-abs (DVE reduce
with apply_absolute_value), shared exponent via int-masked fp32 exponent
bits, q = saturating round-half-even fp32->int8 cast of x * 2^(7-e) (HW cast
verified RNE+saturating), deq = q * 2^(e-7) in bf16 (all deq values are
exactly representable in bf16, so a bf16 matmul with fp32 PSUM accumulation
reproduces the fp32 reference up to accumulation order).
"""
import warnings

warnings.filterwarnings("ignore")
import numpy as np

import concourse.bass as bass
import concourse.mybir as mybir
import concourse.tile as tile
from concourse import bacc
from concourse.bass_utils import run_bass_kernel_spmd

# full problem
B, S, D, F = 4, 2048, 2048, 8192
M_FULL, K, N_FULL = B * S, D, F
# shard grid
RSH, CSH = 4, 2
M = M_FULL // RSH      # 2048 rows / core
N = N_FULL // CSH      # 4096 cols / core
MT = M // 128          # 16 m-tiles
KT = K // 128          # 16 k-tiles
NCH = 8                # n chunks per core
NC_W = N // NCH        # 512 cols per chunk
BS = 32                # bfp block size

F32 = mybir.dt.float32
BF16 = mybir.dt.bfloat16
I32 = mybir.dt.int32
I8 = mybir.dt.int8
RECIP_CONST = float(254 << 23)  # recip_bits = (254<<23) - scale_bits
INV_LN2 = float(np.float32(1.4426950408889634))  # matches device log2 lowering
LN2 = float(np.float32(0.6931471805599453))      # matches device exp2 lowering
MAGIC = 12582912.0  # 1.5 * 2**23 (RNE-to-integer magic)


def _emit_quant(nc, pool, src_f32, dst_bf16, qi8_tile, nblk, tag,
                qpass_pool=False, deq_pool=True):
    """Quantize src_f32 -> dst_bf16 (same shape), blocks of 32 on free axis.

    src/dst views must be [128, nblk*32] contiguous free. qpass_pool/deq_pool
    choose GpSimd vs DVE for the two full-size passes (engine balancing).
    """
    ma = pool.tile([128, nblk], F32, tag=f"{tag}_ma")
    xv = src_f32.rearrange("p (b e) -> p b e", e=BS)
    nc.vector.tensor_reduce(
        ma[:], xv, axis=mybir.AxisListType.X, op=mybir.AluOpType.max,
        apply_absolute_value=True,
    )
    # exponent e = floor(Ln(ma) * (1/ln2)) — replicates the device reference's
    # fp32 log2 (incl. its round-to-integer-crossing behavior) bit-exactly.
    ln_t = pool.tile([128, nblk], F32, tag=f"{tag}_ln")
    nc.scalar.activation(ln_t[:], ma[:], mybir.ActivationFunctionType.Ln)
    t = pool.tile([128, nblk], F32, tag=f"{tag}_t")
    nc.vector.tensor_single_scalar(t[:], ln_t[:], INV_LN2, op=mybir.AluOpType.mult)
    r = pool.tile([128, nblk], F32, tag=f"{tag}_r")
    nc.vector.tensor_scalar(
        r[:], t[:], MAGIC, MAGIC,
        op0=mybir.AluOpType.add, op1=mybir.AluOpType.subtract,
    )
    g = pool.tile([128, nblk], F32, tag=f"{tag}_g")
    nc.vector.tensor_tensor(g[:], r[:], t[:], op=mybir.AluOpType.is_gt)
    e_t = pool.tile([128, nblk], F32, tag=f"{tag}_e")
    nc.vector.tensor_tensor(e_t[:], r[:], g[:], op=mybir.AluOpType.subtract)
    # scale bits = (e + 120) << 23  (fp32 2^(e-7), exact)
    scale_i = pool.tile([128, nblk], I32, tag=f"{tag}_sc")
    nc.vector.tensor_scalar(
        scale_i[:], e_t[:], 120.0, 8388608.0,
        op0=mybir.AluOpType.add, op1=mybir.AluOpType.mult,
    )
    recip_i = pool.tile([128, nblk], I32, tag=f"{tag}_rc")
    nc.vector.tensor_scalar(
        recip_i[:], scale_i[:], RECIP_CONST, -1.0,
        op0=mybir.AluOpType.subtract, op1=mybir.AluOpType.mult,
    )
    # The reference divides by s_dev = device exp2(e-7), which is up to a few
    # hundred ulps off an exact power of two. Replicate: s_dev = Exp(e7*ln2)
    # (bit-matches device exp2), then effective reciprocal
    # r* = 2^(7-e) * (2 - s_dev*2^(7-e)), all exact in fp32, so that
    # RN(x*r*) == RN(x/s_dev) up to ~2^-46 coincidences.
    e7 = pool.tile([128, nblk], F32, tag=f"{tag}_e7")
    nc.vector.tensor_single_scalar(e7[:], e_t[:], 7.0, op=mybir.AluOpType.subtract)
    s_dev = pool.tile([128, nblk], F32, tag=f"{tag}_sd")
    nc.scalar.activation(s_dev[:], e7[:], mybir.ActivationFunctionType.Exp, scale=LN2)
    m_t = pool.tile([128, nblk], F32, tag=f"{tag}_m")
    nc.vector.tensor_tensor(
        m_t[:], s_dev[:], recip_i[:].bitcast(F32), op=mybir.AluOpType.mult,
    )
    c_t = pool.tile([128, nblk], F32, tag=f"{tag}_c")
    nc.vector.tensor_scalar(
        c_t[:], m_t[:], 2.0, -1.0,
        op0=mybir.AluOpType.subtract, op1=mybir.AluOpType.mult,
    )
    rstar = pool.tile([128, nblk], F32, tag=f"{tag}_rs")
    nc.vector.tensor_tensor(
        rstar[:], recip_i[:].bitcast(F32), c_t[:], op=mybir.AluOpType.mult,
    )
    scale_b = scale_i[:].bitcast(F32).unsqueeze(-1).broadcast_to((128, nblk, BS))
    recip_b = rstar[:].unsqueeze(-1).broadcast_to((128, nblk, BS))
    qv = qi8_tile.rearrange("p (b e) -> p b e", e=BS)
    qpass_eng = nc.gpsimd if qpass_pool else nc.vector
    qpass_eng.tensor_tensor(qv, xv, recip_b, op=mybir.AluOpType.mult)
    # dequant product is exact, so any correct int8->fp32->bf16 conversion
    # matches; engine choice is purely load balancing.
    deq_eng = nc.gpsimd if deq_pool else nc.vector
    deq_eng.tensor_tensor(
        dst_bf16.rearrange("p (b e) -> p b e", e=BS), qv, scale_b,
        op=mybir.AluOpType.mult,
    )


def build_nc(with_bias: bool, repeats: int = 1):
    nc = bacc.Bacc(None, target_bir_lowering=False)
    xs = nc.dram_tensor("xs", [M, K], F32, kind="ExternalInput")
    ws = nc.dram_tensor("ws", [K, N], F32, kind="ExternalInput")
    if with_bias:
        bs = nc.dram_tensor("bs", [N], F32, kind="ExternalInput")
    out = nc.dram_tensor("out", [M, N], F32, kind="ExternalOutput")

    with tile.TileContext(nc) as tc:
        with (
            tc.tile_pool(name="xt", bufs=1) as xt_pool,
            tc.tile_pool(name="wq", bufs=4) as wq_pool,
            tc.tile_pool(name="stage", bufs=2) as stage_pool,
            tc.tile_pool(name="xs_p", bufs=2) as xs_pool,
            tc.tile_pool(name="small", bufs=2) as small_pool,
            tc.tile_pool(name="outp", bufs=3) as out_pool,
            tc.tile_pool(name="psum_o", bufs=4, space="PSUM") as psum_o_pool,
            tc.tile_pool(name="const", bufs=1) as const_pool,
        ):
            bias_sb = None
            if with_bias:
                bias_sb = const_pool.tile([128, N], F32)
                bias_bcast = bass.AP(
                    tensor=bs.ap().tensor, offset=bs.ap().offset,
                    ap=[[0, 128], [1, N]],
                )
                nc.sync.dma_start(bias_sb[:], bias_bcast)

            # resident transposed quantized x: per m-tile [128 kpart, KT, 128 m]
            xT = [
                xt_pool.tile([128, KT, 128], BF16, tag=f"xT{m}", name=f"xT{m}")
                for m in range(MT)
            ]

            def emit_x_mtile(m):
                xstage = xs_pool.tile([128, K], F32, tag="xstage")
                nc.sync.dma_start(xstage[:], xs[m * 128:(m + 1) * 128, :])
                xq8 = xs_pool.tile([128, K], I8, tag="xq8", bufs=3)
                xqb = xs_pool.tile([128, K], BF16, tag="xqb", bufs=3)
                _emit_quant(nc, small_pool, xstage[:], xqb[:], xq8[:], K // BS, "x")
                # one big xbar transpose: [128 m, 2048 k] -> [128 kp, KT, 128 m]
                nc.scalar.dma_start_transpose(xT[m][:], xqb[:])

            WH = KT // 4  # k-quarters per w chunk for pipelining

            def emit_w_chunk(c, on_pool=False, pieces=4):
                # independent piece-tiles: MMs for a k-piece only depend on
                # that piece's quant, not the whole chunk
                wh = KT // pieces
                wq_qs = []
                for h in range(pieces):
                    wq_h = wq_pool.tile([128, wh, NC_W], BF16,
                                        tag=f"wq{h % 4}", name=f"wq_c{c}_h{h}")
                    wstage = stage_pool.tile([128, wh, NC_W], F32, tag="wstage")
                    src = bass.AP(
                        tensor=ws.ap().tensor,
                        offset=c * NC_W + h * wh * 128 * N,
                        ap=[[N, 128], [128 * N, wh], [1, NC_W]],
                    )
                    nc.sync.dma_start(wstage[:], src)
                    w8 = stage_pool.tile([128, wh, NC_W], I8, tag="w8")
                    _emit_quant(
                        nc, small_pool,
                        wstage[:].rearrange("p t n -> p (t n)"),
                        wq_h[:].rearrange("p t n -> p (t n)"),
                        w8[:].rearrange("p t n -> p (t n)"),
                        wh * NC_W // BS, "w",
                        qpass_pool=False, deq_pool=True,
                    )
                    wq_qs.append(wq_h)
                return (wq_qs, wh)

            def emit_mm_block(m, c, wq):
                wq_qs, wh = wq
                po = psum_o_pool.tile([128, NC_W], F32, tag="po")
                for k in range(KT):
                    nc.tensor.matmul(
                        po[:], xT[m][:, k, :], wq_qs[k // wh][:, k % wh, :],
                        start=(k == 0), stop=(k == KT - 1),
                    )
                ob = out_pool.tile([128, NC_W], F32, tag="ob")
                if with_bias:
                    nc.vector.tensor_tensor(
                        ob[:], po[:], bias_sb[:, c * NC_W:(c + 1) * NC_W],
                        op=mybir.AluOpType.add,
                    )
                else:
                    nc.scalar.copy(ob[:], po[:])
                nc.sync.dma_start(
                    out[m * 128:(m + 1) * 128, c * NC_W:(c + 1) * NC_W], ob[:],
                )

            def emit_body():
                # x first (short path to first MM), then stream. During the
                # x-feed phase interleave chunks c0/c1/c2 (staggered) so PE
                # demand stays above the x-pipeline's production rate.
                emit_x_mtile(0)
                wq_0 = emit_w_chunk(0, pieces=8)
                emit_x_mtile(1)
                wqs = {0: wq_0}
                for m in range(MT):
                    emit_mm_block(m, 0, wqs[0])
                    if m == 1:
                        wqs[1] = emit_w_chunk(1, on_pool=True)
                    if m == 4:
                        wqs[2] = emit_w_chunk(2, on_pool=True)
                    if m + 2 < MT:
                        emit_x_mtile(m + 2)
                    if m >= 3:
                        emit_mm_block(m - 3, 1, wqs[1])
                    if m >= 6:
                        emit_mm_block(m - 6, 2, wqs[2])
                for m in range(MT - 3, MT):
                    emit_mm_block(m, 1, wqs[1])
                for m in range(MT - 6, MT):
                    emit_mm_block(m, 2, wqs[2])
                for c in range(3, NCH):
                    wq_c = emit_w_chunk(c)
                    for m in range(MT):
                        emit_mm_block(m, c, wq_c)

            if repeats > 1:
                with tc.For_i(0, repeats, 1):
                    emit_body()
            else:
                emit_body()

    nc.finalize()
    return nc


_NC_CACHE = {}


def _get_nc(with_bias: bool):
    if with_bias not in _NC_CACHE:
        _NC_CACHE[with_bias] = build_nc(with_bias)
    return _NC_CACHE[with_bias]


def run(x, kernel, bias, trace=False):
    x2d = np.ascontiguousarray(np.asarray(x, dtype=np.float32).reshape(M_FULL, K))
    w = np.ascontiguousarray(np.asarray(kernel, dtype=np.float32))
    b = np.asarray(bias, dtype=np.float32)
    with_bias = bool(np.any(b))
    nc = _get_nc(with_bias)

    in_maps = []
    for core in range(8):
        r, c = divmod(core, CSH)
        m = {
            "xs": x2d[r * M:(r + 1) * M, :],
            "ws": np.ascontiguousarray(w[:, c * N:(c + 1) * N]),
        }
        if with_bias:
            m["bs"] = np.ascontiguousarray(b[c * N:(c + 1) * N])
        in_maps.append(m)

    res = run_bass_kernel_spmd(nc, in_maps, core_ids=list(range(8)), trace=trace)
    full = np.empty((M_FULL, N_FULL), dtype=np.float32)
    for core in range(8):
        r, c = divmod(core, CSH)
        full[r * M:(r + 1) * M, c * N:(c + 1) * N] = res.results[core]["out"]
    return full.reshape(B, S, N_FULL), res


def kernel(x, kernel, bias):
    out, _ = run(x, kernel, bias, trace=False)
    return out


# revision 41
# speedup vs baseline: 1.0275x; 1.0150x over previous
"""BFP8 dense layer (out = bfp_quant(x) @ bfp_quant(w) + bias) on 8 trn2 cores.

Sharding (hardcoded for x:(4,2048,2048) w:(2048,8192) bias:(8192,)):
  2D tensor-parallel grid: 4 row-shards of x (2048 rows each) x 2 col-shards
  of w (4096 cols each). core = r*2 + c computes out[r*2048:(r+1)*2048,
  c*4096:(c+1)*4096]. Each core quantizes its own shards locally (BFP blocks
  of 32 run along the last axis of both tensors; all shard boundaries are
  multiples of 32, so block structure matches the full-tensor flattening).

Quantization is exact vs the jax reference: per 32-block max-abs (DVE reduce
with apply_absolute_value), shared exponent via int-masked fp32 exponent
bits, q = saturating round-half-even fp32->int8 cast of x * 2^(7-e) (HW cast
verified RNE+saturating), deq = q * 2^(e-7) in bf16 (all deq values are
exactly representable in bf16, so a bf16 matmul with fp32 PSUM accumulation
reproduces the fp32 reference up to accumulation order).
"""
import warnings

warnings.filterwarnings("ignore")
import numpy as np

import concourse.bass as bass
import concourse.mybir as mybir
import concourse.tile as tile
from concourse import bacc
from concourse.bass_utils import run_bass_kernel_spmd

# full problem
B, S, D, F = 4, 2048, 2048, 8192
M_FULL, K, N_FULL = B * S, D, F
# shard grid
RSH, CSH = 4, 2
M = M_FULL // RSH      # 2048 rows / core
N = N_FULL // CSH      # 4096 cols / core
MT = M // 128          # 16 m-tiles
KT = K // 128          # 16 k-tiles
NCH = 8                # n chunks per core
NC_W = N // NCH        # 512 cols per chunk
BS = 32                # bfp block size

F32 = mybir.dt.float32
BF16 = mybir.dt.bfloat16
I32 = mybir.dt.int32
I8 = mybir.dt.int8
RECIP_CONST = float(254 << 23)  # recip_bits = (254<<23) - scale_bits
INV_LN2 = float(np.float32(1.4426950408889634))  # matches device log2 lowering
LN2 = float(np.float32(0.6931471805599453))      # matches device exp2 lowering
MAGIC = 12582912.0  # 1.5 * 2**23 (RNE-to-integer magic)


def _emit_quant(nc, pool, src_f32, dst_bf16, qi8_tile, nblk, tag,
                qpass_pool=False, deq_pool=True):
    """Quantize src_f32 -> dst_bf16 (same shape), blocks of 32 on free axis.

    src/dst views must be [128, nblk*32] contiguous free. qpass_pool/deq_pool
    choose GpSimd vs DVE for the two full-size passes (engine balancing).
    """
    ma = pool.tile([128, nblk], F32, tag=f"{tag}_ma")
    xv = src_f32.rearrange("p (b e) -> p b e", e=BS)
    nc.vector.tensor_reduce(
        ma[:], xv, axis=mybir.AxisListType.X, op=mybir.AluOpType.max,
        apply_absolute_value=True,
    )
    # exponent e = floor(Ln(ma) * (1/ln2)) — replicates the device reference's
    # fp32 log2 (incl. its round-to-integer-crossing behavior) bit-exactly.
    ln_t = pool.tile([128, nblk], F32, tag=f"{tag}_ln")
    nc.scalar.activation(ln_t[:], ma[:], mybir.ActivationFunctionType.Ln)
    t = pool.tile([128, nblk], F32, tag=f"{tag}_t")
    nc.vector.tensor_single_scalar(t[:], ln_t[:], INV_LN2, op=mybir.AluOpType.mult)
    r = pool.tile([128, nblk], F32, tag=f"{tag}_r")
    nc.vector.tensor_scalar(
        r[:], t[:], MAGIC, MAGIC,
        op0=mybir.AluOpType.add, op1=mybir.AluOpType.subtract,
    )
    g = pool.tile([128, nblk], F32, tag=f"{tag}_g")
    nc.vector.tensor_tensor(g[:], r[:], t[:], op=mybir.AluOpType.is_gt)
    e_t = pool.tile([128, nblk], F32, tag=f"{tag}_e")
    nc.vector.tensor_tensor(e_t[:], r[:], g[:], op=mybir.AluOpType.subtract)
    # scale bits = (e + 120) << 23  (fp32 2^(e-7), exact)
    scale_i = pool.tile([128, nblk], I32, tag=f"{tag}_sc")
    nc.vector.tensor_scalar(
        scale_i[:], e_t[:], 120.0, 8388608.0,
        op0=mybir.AluOpType.add, op1=mybir.AluOpType.mult,
    )
    recip_i = pool.tile([128, nblk], I32, tag=f"{tag}_rc")
    nc.vector.tensor_scalar(
        recip_i[:], scale_i[:], RECIP_CONST, -1.0,
        op0=mybir.AluOpType.subtract, op1=mybir.AluOpType.mult,
    )
    # The reference divides by s_dev = device exp2(e-7), which is up to a few
    # hundred ulps off an exact power of two. Replicate: s_dev = Exp(e7*ln2)
    # (bit-matches device exp2), then effective reciprocal
    # r* = 2^(7-e) * (2 - s_dev*2^(7-e)), all exact in fp32, so that
    # RN(x*r*) == RN(x/s_dev) up to ~2^-46 coincidences.
    e7 = pool.tile([128, nblk], F32, tag=f"{tag}_e7")
    nc.vector.tensor_single_scalar(e7[:], e_t[:], 7.0, op=mybir.AluOpType.subtract)
    s_dev = pool.tile([128, nblk], F32, tag=f"{tag}_sd")
    nc.scalar.activation(s_dev[:], e7[:], mybir.ActivationFunctionType.Exp, scale=LN2)
    m_t = pool.tile([128, nblk], F32, tag=f"{tag}_m")
    nc.vector.tensor_tensor(
        m_t[:], s_dev[:], recip_i[:].bitcast(F32), op=mybir.AluOpType.mult,
    )
    c_t = pool.tile([128, nblk], F32, tag=f"{tag}_c")
    nc.vector.tensor_scalar(
        c_t[:], m_t[:], 2.0, -1.0,
        op0=mybir.AluOpType.subtract, op1=mybir.AluOpType.mult,
    )
    rstar = pool.tile([128, nblk], F32, tag=f"{tag}_rs")
    nc.vector.tensor_tensor(
        rstar[:], recip_i[:].bitcast(F32), c_t[:], op=mybir.AluOpType.mult,
    )
    scale_b = scale_i[:].bitcast(F32).unsqueeze(-1).broadcast_to((128, nblk, BS))
    recip_b = rstar[:].unsqueeze(-1).broadcast_to((128, nblk, BS))
    qv = qi8_tile.rearrange("p (b e) -> p b e", e=BS)
    qpass_eng = nc.gpsimd if qpass_pool else nc.vector
    qpass_eng.tensor_tensor(qv, xv, recip_b, op=mybir.AluOpType.mult)
    # dequant product is exact, so any correct int8->fp32->bf16 conversion
    # matches; engine choice is purely load balancing.
    deq_eng = nc.gpsimd if deq_pool else nc.vector
    deq_eng.tensor_tensor(
        dst_bf16.rearrange("p (b e) -> p b e", e=BS), qv, scale_b,
        op=mybir.AluOpType.mult,
    )


def build_nc(with_bias: bool, repeats: int = 1):
    nc = bacc.Bacc(None, target_bir_lowering=False)
    xs = nc.dram_tensor("xs", [M, K], F32, kind="ExternalInput")
    ws = nc.dram_tensor("ws", [K, N], F32, kind="ExternalInput")
    if with_bias:
        bs = nc.dram_tensor("bs", [N], F32, kind="ExternalInput")
    out = nc.dram_tensor("out", [M, N], F32, kind="ExternalOutput")

    with tile.TileContext(nc) as tc:
        with (
            tc.tile_pool(name="xt", bufs=1) as xt_pool,
            tc.tile_pool(name="wq", bufs=4) as wq_pool,
            tc.tile_pool(name="stage", bufs=2) as stage_pool,
            tc.tile_pool(name="xs_p", bufs=2) as xs_pool,
            tc.tile_pool(name="small", bufs=2) as small_pool,
            tc.tile_pool(name="outp", bufs=3) as out_pool,
            tc.tile_pool(name="psum_o", bufs=8, space="PSUM") as psum_o_pool,
            tc.tile_pool(name="const", bufs=1) as const_pool,
        ):
            bias_sb = None
            if with_bias:
                bias_sb = const_pool.tile([128, N], F32)
                bias_bcast = bass.AP(
                    tensor=bs.ap().tensor, offset=bs.ap().offset,
                    ap=[[0, 128], [1, N]],
                )
                nc.sync.dma_start(bias_sb[:], bias_bcast)

            # resident transposed quantized x: per m-tile [128 kpart, KT, 128 m]
            xT = [
                xt_pool.tile([128, KT, 128], BF16, tag=f"xT{m}", name=f"xT{m}")
                for m in range(MT)
            ]

            def emit_x_mtile(m):
                xstage = xs_pool.tile([128, K], F32, tag="xstage")
                nc.sync.dma_start(xstage[:], xs[m * 128:(m + 1) * 128, :])
                xq8 = xs_pool.tile([128, K], I8, tag="xq8", bufs=3)
                xqb = xs_pool.tile([128, K], BF16, tag="xqb", bufs=3)
                _emit_quant(nc, small_pool, xstage[:], xqb[:], xq8[:], K // BS, "x")
                # one big xbar transpose: [128 m, 2048 k] -> [128 kp, KT, 128 m]
                nc.scalar.dma_start_transpose(xT[m][:], xqb[:])

            WH = KT // 4  # k-quarters per w chunk for pipelining

            def emit_w_chunk(c, on_pool=False, pieces=4):
                # independent piece-tiles: MMs for a k-piece only depend on
                # that piece's quant, not the whole chunk
                wh = KT // pieces
                wq_qs = []
                for h in range(pieces):
                    wq_h = wq_pool.tile([128, wh, NC_W], BF16,
                                        tag=f"wq{h % 4}", name=f"wq_c{c}_h{h}")
                    wstage = stage_pool.tile([128, wh, NC_W], F32, tag="wstage")
                    src = bass.AP(
                        tensor=ws.ap().tensor,
                        offset=c * NC_W + h * wh * 128 * N,
                        ap=[[N, 128], [128 * N, wh], [1, NC_W]],
                    )
                    nc.sync.dma_start(wstage[:], src)
                    w8 = stage_pool.tile([128, wh, NC_W], I8, tag="w8")
                    _emit_quant(
                        nc, small_pool,
                        wstage[:].rearrange("p t n -> p (t n)"),
                        wq_h[:].rearrange("p t n -> p (t n)"),
                        w8[:].rearrange("p t n -> p (t n)"),
                        wh * NC_W // BS, "w",
                        qpass_pool=False, deq_pool=True,
                    )
                    wq_qs.append(wq_h)
                return (wq_qs, wh)

            def emit_mm_block(m, c, wq):
                wq_qs, wh = wq
                po = psum_o_pool.tile([128, NC_W], F32, tag="po")
                for k in range(KT):
                    nc.tensor.matmul(
                        po[:], xT[m][:, k, :], wq_qs[k // wh][:, k % wh, :],
                        start=(k == 0), stop=(k == KT - 1),
                    )
                ob = out_pool.tile([128, NC_W], F32, tag="ob")
                if with_bias:
                    nc.vector.tensor_tensor(
                        ob[:], po[:], bias_sb[:, c * NC_W:(c + 1) * NC_W],
                        op=mybir.AluOpType.add,
                    )
                else:
                    nc.scalar.copy(ob[:], po[:])
                nc.sync.dma_start(
                    out[m * 128:(m + 1) * 128, c * NC_W:(c + 1) * NC_W], ob[:],
                )

            def emit_body():
                # x first (short path to first MM), then stream. During the
                # x-feed phase interleave chunks c0/c1/c2 (staggered) so PE
                # demand stays above the x-pipeline's production rate.
                emit_x_mtile(0)
                wq_0 = emit_w_chunk(0, pieces=8)
                emit_x_mtile(1)
                wqs = {0: wq_0}
                for m in range(MT):
                    emit_mm_block(m, 0, wqs[0])
                    if m == 1:
                        wqs[1] = emit_w_chunk(1)
                    if m == 4:
                        wqs[2] = emit_w_chunk(2)
                    if m + 2 < MT:
                        emit_x_mtile(m + 2)
                    if m >= 3:
                        emit_mm_block(m - 3, 1, wqs[1])
                    if m >= 6:
                        emit_mm_block(m - 6, 2, wqs[2])
                for m in range(MT - 3, MT):
                    emit_mm_block(m, 1, wqs[1])
                for m in range(MT - 6, MT):
                    emit_mm_block(m, 2, wqs[2])
                for c in range(3, NCH):
                    wq_c = emit_w_chunk(c)
                    for m in range(MT):
                        emit_mm_block(m, c, wq_c)

            if repeats > 1:
                with tc.For_i(0, repeats, 1):
                    emit_body()
            else:
                emit_body()

    nc.finalize()
    return nc


_NC_CACHE = {}


def _get_nc(with_bias: bool):
    if with_bias not in _NC_CACHE:
        _NC_CACHE[with_bias] = build_nc(with_bias)
    return _NC_CACHE[with_bias]


def run(x, kernel, bias, trace=False):
    x2d = np.ascontiguousarray(np.asarray(x, dtype=np.float32).reshape(M_FULL, K))
    w = np.ascontiguousarray(np.asarray(kernel, dtype=np.float32))
    b = np.asarray(bias, dtype=np.float32)
    with_bias = bool(np.any(b))
    nc = _get_nc(with_bias)

    in_maps = []
    for core in range(8):
        r, c = divmod(core, CSH)
        m = {
            "xs": x2d[r * M:(r + 1) * M, :],
            "ws": np.ascontiguousarray(w[:, c * N:(c + 1) * N]),
        }
        if with_bias:
            m["bs"] = np.ascontiguousarray(b[c * N:(c + 1) * N])
        in_maps.append(m)

    res = run_bass_kernel_spmd(nc, in_maps, core_ids=list(range(8)), trace=trace)
    full = np.empty((M_FULL, N_FULL), dtype=np.float32)
    for core in range(8):
        r, c = divmod(core, CSH)
        full[r * M:(r + 1) * M, c * N:(c + 1) * N] = res.results[core]["out"]
    return full.reshape(B, S, N_FULL), res


def kernel(x, kernel, bias):
    out, _ = run(x, kernel, bias, trace=False)
    return out
